# revision 40
# baseline (speedup 1.0000x reference)
"""Trainium2 Bass kernel: single-head causal attention.

B=4, T=4096, E=512, H=64, fp32 in/out.

Sharding: 2 cores per batch sample, split by keys (even/odd 128-strips via
a per-256-block half rotation baked in on the host). Each core computes a
partial softmax (numerator + denominator via a ones-column in V) over its
half of the keys for all 4096 queries; the host combines
out = (num0+num1)/(den0+den1).

Device kernel (per core):
  - x streams on the sync queue: quarter 0 per-e-strip (4 DMAs so the
    kv/q projection matmuls pipeline with arrival), quarters 1-3 as
    single 1MB DMAs (DMA-issue instructions cost ~600ns each on the
    sequencer). Small tensors (weights, biases, masks) ride the scalar
    HWDGE queue in parallel.
  - HAM clock-gate ramp: junk matmuls (PSS-pool scratch tiles) run from
    the end of the engine preamble until the first x strip lands, and
    fill the per-strip arrival gaps, so the K=4/8 -> 8/8 un-throttle
    fires ~11us in instead of ~21us. Bare LDWEIGHTS do NOT count as HAM
    activity - only matmuls do.
  - Q projection uses [Wq|Wq] stationary so PSUM rows 0:64 and 64:128
    both hold Q -- the partition-64:128 copy feeds row-tiled scores.
  - K^T duplicates at partitions 64:128 (kd): DVE copy for the first two
    kv chunks (the gpsimd software-DGE queue has multi-us latency and
    would starve chunk-2/3 scores), gpsimd SBUF DMA for the rest.
  - Scores (contraction H=64) run as two concurrent row-tiled matmuls
    (tile_position (0,0)/(64,0) auto-derived from base partitions).
  - PV split into two 64-row matmuls (key halves) accumulating into two
    PSUM banks; halves summed on the host. Whole attention stream stays
    in 64-row PE tiling mode (no mode-switch drains).
  - kv chunk 0's V^T->V PE transposes are DEFERRED until after the first
    score pair: they are not needed until the first PV, and inline they
    add ~2.5us (cold clock) to the x->first-exp critical path.
  - Output partials evacuated as fp16 with a 2^-6 scale folded in (the
    scale cancels in the host's num/den division).
  - Diagonal trim: 768 of 1024 score columns computed for the diag pair.
  - exp on the scalar engine with fused 1/sqrt(H) scale (no max
    subtraction; scores are bounded); ~7 late-chunk pairs offload their
    exp to the DVE (single-pass int16 Schraudolph, PV reads the bits as
    bf16) to compress the pacing scalar stream.
  - The DIAGONAL pair goes LAST in each chunk: it is the only pair that
    needs the chunk's newest data (its own x half, kd dup, V strips), so
    each chunk starts on old strips + Q(c) alone and the mask multiply
    leaves the chunk-start critical path. PV emission order additionally
    delays each SCH pair's PV one slot (accumulation start/stop flags
    follow emission order).
  - Last chunk evacuates on the scalar engine (idle after the final exp).
"""

import functools

import numpy as np
import ml_dtypes

B, T, E, H = 4, 4096, 512, 64
NCORES = 8
NCHUNK = 8  # 512-query chunks per sample
CHUNK = T // NCHUNK  # 512
NSTRIP = 16  # local 128-key strips per core (half of T/128)
VSTRIDE = 80  # per-strip stride in the packed V tile
NWARM = 8  # junk matmuls before the first real matmul (HAM clock ramp)
# non-diag pairs whose exp runs on the DVE via a single-pass int16
# Schraudolph (bf16 bits = int16(s*a+b); ~3.3% max elem err, washed by
# the softmax normalization) -- relieves the pacing scalar engine in the
# exp-heavy late chunks
SCH_OFFLOAD = {(4, 2), (5, 3), (6, 1), (6, 3), (7, 1), (7, 3), (7, 5)}
SCH_MUL16 = 12102203.16 / 8.0 / 65536.0  # log2(e)*2^23 * (1/sqrt(64)) >> 16
SCH_ADD16 = (127 * 2**23 - 367500.0) / 65536.0  # bias calibrated vs np.exp
PACKED_FROM = 2  # chunks >= this use row-tiled scores (kd ready by then)

bf16 = ml_dtypes.bfloat16


@functools.lru_cache(maxsize=1)
def _build():
    import concourse.mybir as mybir
    from concourse import bacc
    import concourse.tile as tile
    from concourse.masks import make_identity

    dt_bf = mybir.dt.bfloat16
    dt_f32 = mybir.dt.float32

    nc = bacc.Bacc("TRN2", target_bir_lowering=False, num_devices=NCORES)

    # x^T, rotated, (quarter, e-strip)-blocked: [4, 128, 4, 1024]
    xt = nc.dram_tensor("xt", [4, 128, 4, T // 4], dt_bf, kind="ExternalInput")
    wq2 = nc.dram_tensor("wq2", [128, 4 * 128], dt_bf, kind="ExternalInput")
    wkv = nc.dram_tensor("wkv", [128, 4 * 128], dt_bf, kind="ExternalInput")
    bias_q2 = nc.dram_tensor("bias_q2", [128, 1], dt_f32, kind="ExternalInput")
    bias_kv = nc.dram_tensor("bias_kv", [128, 1], dt_f32, kind="ExternalInput")
    masks = nc.dram_tensor("masks", [128, 768], dt_bf, kind="ExternalInput")
    # per chunk: [key-half-0 partial | key-half-1 partial], host adds them.
    # fp16 with a 2^-6 scale folded in (cancels in the host's num/den).
    dt_f16 = mybir.dt.float16
    out_d = nc.dram_tensor("out", [H + 1, 2 * T], dt_f16, kind="ExternalOutput")

    scale = 1.0 / float(np.sqrt(H))

    with tile.TileContext(nc) as tc:
        with (
            tc.tile_pool(name="const", bufs=1) as cpool,
            tc.tile_pool(name="xt_pool", bufs=1) as xpool,
            tc.tile_pool(name="q_pool", bufs=3) as qpool,
            tc.tile_pool(name="kv_pool", bufs=4) as kvpool,
            tc.tile_pool(name="kd_pool", bufs=4) as kdpool,
            tc.tile_pool(name="v_pool", bufs=1) as vpool,
            tc.tile_pool(name="p_pool", bufs=4) as ppool,
            tc.tile_pool(name="o_pool", bufs=2) as opool,
            tc.tile_pool(name="ps_proj", bufs=2, space="PSUM") as pspr,
            tc.tile_pool(name="ps_s", bufs=2, space="PSUM") as pss,
            tc.tile_pool(name="ps_o", bufs=1, space="PSUM") as pso,
        ):
            # ---- input DMAs ----
            xt_sb = xpool.tile([128, 4 * T], dt_bf)

            def xt_block(qd, es):
                off = (qd * 4 + es) * 1024
                return xt_sb[:, off : off + 1024]

            wkv_sb = cpool.tile([128, 512], dt_bf)
            wq2_sb = cpool.tile([128, 512], dt_bf)
            nc.sync.dma_start(wkv_sb, wkv.ap())

            # x streams as 8 x 512KB half-quarter DMAs (tokens h*512 of
            # each quarter, strided across the 4 e-strips). One DMA per
            # 512-token half: chunk c (and its 2 new key strips) is fully
            # enabled by half-DMA c. Small DMAs serialize at ~1.4us each
            # on the queue regardless of size, so this is the sweet spot.
            def xh_dma(i):
                qd, h = i // 2, i % 2
                dst = xt_sb[:, qd * 4096 : (qd + 1) * 4096].rearrange(
                    "p (a t) -> p a t", t=1024
                )[:, :, h * 512 : (h + 1) * 512]
                nc.sync.dma_start(dst, xt.ap()[qd][:, :, h * 512 : (h + 1) * 512])

            xh_dma(0)
            # wq2 rides the sync queue after xh0 (q0's matmuls start ~2us
            # after kv0's): on the scalar HWDGE queue it lands ~15.5us
            # and blocks the Q projection
            nc.sync.dma_start(wq2_sb, wq2.ap())
            for i in range(1, 8):
                xh_dma(i)
            bkv_sb = cpool.tile([128, 1], dt_f32)
            nc.scalar.dma_start(bkv_sb, bias_kv.ap())
            bq2_sb = cpool.tile([128, 1], dt_f32)
            nc.scalar.dma_start(bq2_sb, bias_q2.ap())
            masks_sb = cpool.tile([128, 768], dt_bf)
            nc.scalar.dma_start(masks_sb, masks.ap())

            # ---- HAM warm-up: junk matmuls on rotating PSS-pool scratch
            # tiles (safe to emit anywhere - no readers) ----
            zt = cpool.tile([128, 512], dt_bf)
            nc.gpsimd.memset(zt, 0.0)
            ident = cpool.tile([128, 128], dt_bf)
            make_identity(nc, ident)

            def junk(n):
                jt = pss.tile([128, 512], dt_f32, tag="pss")
                for _ in range(n):
                    nc.tensor.matmul(
                        jt, lhsT=zt[:, 0:128], rhs=zt, start=True, stop=True
                    )

            junk(NWARM)

            # packed V (natural [k,h] layout + ones column for denominator)
            v_nat = vpool.tile([128, NSTRIP * VSTRIDE], dt_bf)
            v3 = v_nat.rearrange("p (s c) -> p s c", c=VSTRIDE)
            nc.vector.memset(v3[:, :, 64:65], 1.0)

            kv_tiles = []
            kd_tiles = []
            q_tiles = []

            def v_transpose(ckv, j):
                # V^T block -> natural V strip via PE transpose
                s = 4 * ckv + j
                kv_sb = kv_tiles[ckv]
                ps_tr = pspr.tile([128, 128], dt_bf, tag="proj")
                nc.tensor.transpose(ps_tr, kv_sb[:, j * 128 : (j + 1) * 128], ident)
                nc.vector.tensor_copy(
                    v_nat[:, s * VSTRIDE : s * VSTRIDE + 64],
                    ps_tr[:, 64:128],
                )

            def kv_alloc():
                kv_sb = kvpool.tile([128, 512], dt_bf, tag="kv")
                kd = kdpool.tile([128, 512], dt_bf, tag="kd")
                kv_tiles.append(kv_sb)
                kd_tiles.append(kd)
                return kv_sb, kd

            def kv_proj_half(ckv, h, defer_tr=False):
                """Project key strips 2*ckv*2+2h, +1 from quarter-ckv
                tokens [h*512:(h+1)*512] (one half-quarter x DMA)."""
                kv_sb, kd = kv_tiles[ckv], kd_tiles[ckv]
                ps_kvh = pspr.tile([128, 256], dt_f32, tag="proj")
                for es in range(4):
                    blk = xt_block(ckv, es)[:, h * 512 : (h + 1) * 512]
                    key_rhs = blk.rearrange(
                        "p (a two b) -> p a two b", two=2, b=128
                    )[:, :, 0, :]
                    nc.tensor.matmul(
                        ps_kvh,
                        lhsT=wkv_sb[:, es * 128 : (es + 1) * 128],
                        rhs=key_rhs,
                        start=(es == 0),
                        stop=(es == 3),
                    )
                c0, c1 = h * 256, (h + 1) * 256
                nc.vector.tensor_scalar_add(kv_sb[:, c0:c1], ps_kvh, bkv_sb)
                # K^T duplicate at partitions 64:128 for row-tiled scores
                # (DVE region copy; the gpsimd software-DGE moves ~8GB/s
                # and would starve the row-tiled scores)
                nc.vector.tensor_copy(kd[64:128, c0:c1], kv_sb[0:64, c0:c1])
                # V^T -> V strips: PE transpose early (deferred for kv
                # chunk 0), DMA xbar late
                for j in (2 * h, 2 * h + 1):
                    if ckv < 2:
                        if not defer_tr:
                            v_transpose(ckv, j)
                    else:
                        s = 4 * ckv + j
                        nc.sync.dma_start(
                            v_nat[:, s * VSTRIDE : s * VSTRIDE + 64],
                            kv_sb[64:128, j * 128 : (j + 1) * 128],
                            transpose=True,
                        )

            def q_proj(c):
                ps_q = pspr.tile([128, 512], dt_f32, tag="proj")
                for es in range(4):
                    nc.tensor.matmul(
                        ps_q,
                        lhsT=wq2_sb[:, es * 128 : (es + 1) * 128],
                        rhs=xt_block(c // 2, es)[
                            :, (c % 2) * CHUNK : (c % 2) * CHUNK + CHUNK
                        ],
                        start=(es == 0),
                        stop=(es == 3),
                    )
                q_sb = qpool.tile([128, 512], dt_bf, tag="q")
                nc.vector.tensor_scalar_add(q_sb, ps_q, bq2_sb)
                q_tiles.append(q_sb)

            def emit_S(c, g):
                """Scores for strip pair g of chunk c: strip 2g (512 query
                cols) and strip 2g+1 (256 cols if diagonal, else 512)."""
                diag = g == c
                w2 = 256 if diag else 512
                ps = pss.tile([128, 1024], dt_f32, tag="pss")
                q = q_tiles[c]
                l0, l1 = 2 * g, 2 * g + 1
                lt0 = kv_tiles[l0 // 4][0:64, (l0 % 4) * 128 : (l0 % 4 + 1) * 128]
                if c >= PACKED_FROM:
                    # concurrent row-tiled pair: (0,0) and (64,0)
                    lt1 = kd_tiles[l1 // 4][64:128, (l1 % 4) * 128 : (l1 % 4 + 1) * 128]
                    r1 = q[64:128, 512 - w2 : 512]
                else:
                    lt1 = kv_tiles[l1 // 4][0:64, (l1 % 4) * 128 : (l1 % 4 + 1) * 128]
                    r1 = q[0:64, 512 - w2 : 512]
                nc.tensor.matmul(
                    ps[:, 0:512], lhsT=lt0, rhs=q[0:64, :], start=True, stop=True
                )
                nc.tensor.matmul(
                    ps[:, 512 : 512 + w2], lhsT=lt1, rhs=r1, start=True, stop=True
                )
                return ps

            def emit_E(c, g, ps):
                diag = g == c
                if (c, g) in SCH_OFFLOAD:
                    # exp on the DVE: bf16 bits = int16(s*a + b) (single-
                    # pass int16 Schraudolph); PV reads the bits as bf16
                    ib = ppool.tile([128, 1024], mybir.dt.int16, tag="p")
                    nc.vector.tensor_scalar(
                        ib,
                        ps[:, 0:1024],
                        SCH_MUL16,
                        SCH_ADD16,
                        mybir.AluOpType.mult,
                        mybir.AluOpType.add,
                    )
                    return ib[:, :].bitcast(dt_bf)
                w = 768 if diag else 1024
                p = ppool.tile([128, 1024], dt_bf, tag="p")
                nc.scalar.activation(
                    p[:, 0:w],
                    ps[:, 0:w],
                    mybir.ActivationFunctionType.Exp,
                    scale=scale,
                )
                if diag:
                    nc.vector.tensor_mul(p[:, 0:768], p[:, 0:768], masks_sb)
                return p

            def emit_V(c, g, p, pso_t, first, last):
                """PV for strip pair g, split into key halves h0/h1 (two
                concurrent 64-row matmuls into separate PSUM banks)."""
                diag = g == c
                w2 = 256 if diag else 512
                for i, (l, pc0, pc1, oc0) in enumerate(
                    (
                        (2 * g, 0, 512, 0),
                        (2 * g + 1, 512, 512 + w2, 512 - w2),
                    )
                ):
                    start = first and i == 0
                    stop = last and i == 1
                    vs = v_nat[:, l * VSTRIDE : l * VSTRIDE + 65]
                    nc.tensor.matmul(
                        pso_t[:, oc0:512],
                        lhsT=vs[0:64, :],
                        rhs=p[0:64, pc0:pc1],
                        start=start,
                        stop=stop,
                    )
                    nc.tensor.matmul(
                        pso_t[:, 512 + oc0 : 1024],
                        lhsT=vs[64:128, :],
                        rhs=p[64:128, pc0:pc1],
                        start=start,
                        stop=stop,
                    )

            def emit_O(c, pso_t):
                # single-PSUM-input copy; the host adds the two key-half
                # partials. The last chunk evacuates in HALVES on the
                # scalar (idle after the final exp) and vector engines in
                # parallel, each half's DMA issued as it lands - shaves
                # the serial tail.
                o = opool.tile([H + 1, 1024], dt_f16, tag="o")
                if c == NCHUNK - 1:
                    nc.scalar.activation(
                        o[:, 0:512],
                        pso_t[:, 0:512],
                        mybir.ActivationFunctionType.Copy,
                        scale=2.0**-6,
                    )
                    nc.sync.dma_start(
                        out_d.ap()[:, c * 1024 : c * 1024 + 512], o[:, 0:512]
                    )
                    nc.vector.tensor_scalar_mul(
                        o[:, 512:1024], pso_t[:, 512:1024], 2.0**-6
                    )
                    nc.sync.dma_start(
                        out_d.ap()[:, c * 1024 + 512 : (c + 1) * 1024], o[:, 512:1024]
                    )
                else:
                    nc.vector.tensor_scalar_mul(o, pso_t, 2.0**-6)
                    nc.sync.dma_start(out_d.ap()[:, c * 1024 : (c + 1) * 1024], o)

            def proj_filler(c):
                # emitted after the diag exp of chunk c but BEFORE the
                # chunk's PV tail (which is gated on the mask via the
                # DVE): half-quarter x DMA c+2 lands 2-5us before this
                # point, so the projections for chunk c+2 overlap the
                # current chunk's exps instead of trailing its last PV.
                # Q first: chunk c+2 opens on old strips + Q(c+2) alone.
                if c > 5:
                    return
                ckv, h = (c + 2) // 2, (c + 2) % 2
                if h == 0:
                    kv_alloc()
                q_proj(c + 2)
                kv_proj_half(ckv, h)

            # ---- software-pipelined main loop: scores run two pairs
            # ahead of exp; PV trails exp by one pair. The DIAGONAL pair
            # goes FIRST in each chunk so the masked PV (the only one
            # gated on the vector engine) is off the chunk-tail chain ----
            all_pairs = [
                (c, g)
                for c in range(NCHUNK)
                for g in (list(range(c)) + [c])
            ]
            ps_map = {}
            sptr = 0

            def pump_S(n):
                nonlocal sptr
                for _ in range(n):
                    if sptr < len(all_pairs):
                        cc, gg = all_pairs[sptr]
                        ps_map[(cc, gg)] = emit_S(cc, gg)
                        sptr += 1

            # chunk-0/1 prologue: strips 0-1 + Q(0) need only the first
            # 512 tokens of quarter 0 -> S(0,0) is emitted after just 8
            # matmuls; the chunk-1 projections follow it, not precede it
            kv_alloc()
            kv_proj_half(0, 0, defer_tr=True)
            # junk bursts between prologue groups keep the PE stream
            # gap-free through the x/bias waits: the free-running 3.4us
            # HAM window then ALWAYS sees a full busy window by ~14.5us
            # (with only the upfront junk, ~1/3 of runs miss the window
            # phase and stay cold to ~20us). The S tiles are pumped
            # AFTER all junk so the pss-pool rotation never makes a junk
            # group wait on a live score tile.
            junk(1)
            q_proj(0)
            junk(1)
            # S(0,0) needs only kv0A + q0 -> pump it ~2us before kv0B.
            # Slot rotation: this S tile lands on the slot of the junk
            # group before q0 (no readers), and the junk group after q1
            # lands on the slot of the one before it - never on a live
            # score tile.
            pump_S(1)
            # q1 before the kv0 second half: S(1,0) (chunk 1's opener)
            # needs only Q(1) + old strips; strips 2,3 feed the diag
            q_proj(1)
            junk(1)
            kv_proj_half(0, 1, defer_tr=True)
            pump_S(1)
            # deferred kv-chunk-0 V transposes: needed by the first PVs,
            # but off the x -> first-exp critical path
            for j in range(4):
                v_transpose(0, j)
            for c in range(NCHUNK):
                # DIAG pair goes LAST: it is the only pair needing the
                # chunk's NEWEST data (x half c's key strips, kd, V
                # strips) - the chunk starts on old strips + Q(c) alone,
                # and the mask multiply leaves the chunk-start path
                e_order = list(range(c)) + [c]
                # PV emission order: an SCH pair's PV is swapped one slot
                # later so the in-order PE stream doesn't stall on the
                # DVE exp (which gets ~2 exp-slots of latency)
                v_order = list(e_order)
                for g in e_order:
                    if (c, g) in SCH_OFFLOAD:
                        i = v_order.index(g)
                        if i + 1 < len(v_order):
                            v_order[i], v_order[i + 1] = v_order[i + 1], v_order[i]
                pso_t = pso.tile([H + 1, 1024], dt_f32, tag="pso")
                p_ready = {}
                vptr = 0
                for g in e_order:
                    p_ready[g] = emit_E(c, g, ps_map.pop((c, g)))
                    pump_S(1)
                    if g == c:
                        proj_filler(c)
                    while vptr < len(v_order) and v_order[vptr] in p_ready:
                        g2 = v_order[vptr]
                        emit_V(
                            c,
                            g2,
                            p_ready.pop(g2),
                            pso_t,
                            vptr == 0,
                            vptr == len(v_order) - 1,
                        )
                        vptr += 1
                emit_O(c, pso_t)

    nc.compile()
    return nc


def _perm(rho):
    """Rotated-order permutation: rotated position i holds original token
    perm[i]. Involutive (half swap within each 256-block)."""
    i = np.arange(T)
    return (i // 256) * 256 + ((i % 256) + 128 * rho) % 256


def _make_in_maps(x, Wq, bq, Wk, bk, Wv, bv):
    # [Wq|Wq] per e-strip: the partition-64:128 copy of Q feeds the
    # row-tiled score matmuls.
    wq4 = Wq.reshape(4, 128, 64)
    wq2_pack = np.ascontiguousarray(
        np.concatenate([wq4, wq4], axis=2).transpose(1, 0, 2).reshape(128, 512)
    ).astype(bf16)
    wkv_pack = np.ascontiguousarray(
        np.concatenate([Wk.reshape(4, 128, 64), Wv.reshape(4, 128, 64)], axis=2)
        .transpose(1, 0, 2)
        .reshape(128, 512)
    ).astype(bf16)
    bias_q2 = np.ascontiguousarray(
        np.concatenate([bq, bq])[:, None]
    ).astype(np.float32)
    bias_kv = np.ascontiguousarray(np.concatenate([bk, bv])[:, None]).astype(
        np.float32
    )

    kk = np.arange(128)[:, None]
    in_maps = []
    for b in range(B):
        xt_b = np.ascontiguousarray(x[b].T).astype(bf16).reshape(4, 128, T)
        for rho in range(2):
            perm = _perm(rho)
            xt_rot = xt_b[:, :, perm]  # rotated token order
            xt_in = np.ascontiguousarray(
                xt_rot.reshape(4, 128, 4, T // 4).transpose(2, 1, 0, 3)
            )
            # masks: columns in rotated order; v = original within-chunk
            # offset of rotated column j (chunk-independent). m1 is zero
            # on query cols 0:256 for both cores -> only cols 256:512 kept.
            v = perm[:CHUNK]
            m0 = (kk - v[None, :] <= -128 * rho).astype(bf16)
            m1 = (kk - v[None, :] <= -256 - 128 * rho).astype(bf16)
            masks_np = np.ascontiguousarray(
                np.concatenate([m0, m1[:, 256:512]], axis=1)
            )
            in_maps.append(
                {
                    "xt": xt_in,
                    "wq2": wq2_pack,
                    "wkv": wkv_pack,
                    "bias_q2": bias_q2,
                    "bias_kv": bias_kv,
                    "masks": masks_np,
                }
            )
    return in_maps


def _combine(results):
    out = np.empty((B, T, H), np.float32)
    p1 = _perm(1)
    for b in range(B):
        # fold the two key-half partials: [65, 8, 2, 512] -> [65, 4096]
        a0 = (
            results[2 * b]["out"]
            .astype(np.float64)
            .reshape(H + 1, NCHUNK, 2, CHUNK)
            .sum(axis=2)
            .reshape(H + 1, T)
        )
        a1 = (
            results[2 * b + 1]["out"]
            .astype(np.float64)
            .reshape(H + 1, NCHUNK, 2, CHUNK)
            .sum(axis=2)
            .reshape(H + 1, T)
        )
        a1 = a1[:, p1]  # un-rotate core-1 columns (involutive perm)
        num = a0[:H] + a1[:H]
        den = a0[H] + a1[H]
        out[b] = (num / den).T.astype(np.float32)
    return out


def _run(trace=False, **inputs):
    from concourse import bass_utils

    nc = _build()
    in_maps = _make_in_maps(
        np.asarray(inputs["x"], np.float32),
        np.asarray(inputs["Wq"], np.float32),
        np.asarray(inputs["bq"], np.float32),
        np.asarray(inputs["Wk"], np.float32),
        np.asarray(inputs["bk"], np.float32),
        np.asarray(inputs["Wv"], np.float32),
        np.asarray(inputs["bv"], np.float32),
    )
    res = bass_utils.run_bass_kernel_spmd(
        nc, in_maps, list(range(NCORES)), trace=trace
    )
    return _combine(res.results), res.exec_time_ns


def kernel(**inputs):
    out, _ = _run(trace=False, **inputs)
    return out
